# revision 1
# baseline (speedup 1.0000x reference)
"""Trainium2 Bass kernel for nn_ContrastiveLoss (SimCLR-style, N=8192, D=128).

v4: pair-symmetric sharding + contiguous DMA + bf16 psum + lean prologue.

Sharding: sim is symmetric, so each unordered pair {i,j} needs ONE exp.
Core c gets z rotated by -c*1024 rows, truncated to 5120 rows: its own
1024 rows plus the next 4 block-columns.  It computes
e = exp(10*cos - 10) for rows x all 5120 local columns:
  - row sums (ACT accum_out)      -> partial S for its OWN rows
  - col sums over local cols 1024..4095 (PE ones-matmul) -> partial S
    for rows owned by cores c+1..c+3.  Cols 0..1023 (own diag block,
    computed in full) and 4096..5119 (the {c,c+4} block, computed by
    BOTH endpoints, row sums only) need no colsum.
  - sum_r pos-cosine via a DVE dot (ln e_pos = 10 g_pos - 10 exactly).
Host assembles S_r = rowsum + colsums - 1 (diag), then
loss = mean(ln S_r + 10 - 10 g_pos_r).  ACT (1 exp/cycle/lane @1.2GHz)
is the bottleneck engine; exp work per core is 0.625x full-matrix.

v4 vs v3:
  - input passed host-pre-tiled [128, 40, 128] so every DMA descriptor
    reads contiguous DRAM (the old (n p) d rearrange walked 512B reads
    at 64KB stride -> ~80GB/s; this one streams at full rate)
  - squares on ACT (Square lives in the SAME table set as Exp -> still
    one table load); DVE does only reduce + rsqrt + scale in the
    prologue; rsqrt = fp32 bit-trick seed + 2 Newton steps (DVE only)
  - sim matmuls write BF16 psum: 1024-wide moving operands, half the
    matmul+ldweights count; exp reads bf16 psum (cos_rr = 1 still
    rounds to exactly 1.0, verified: end-to-end err ~1e-5)
  - separate psum pools for matmul chunks / transposes / colsums so the
    slot rotation never couples the PE transpose+colsum work to the
    ACT-paced chunk stream
"""

import sys

sys.path.insert(0, "/opt/trn_rl_repo")

from contextlib import ExitStack

import numpy as np

import concourse.bass as bass
import concourse.bacc as bacc
import concourse.tile as tile
from concourse import mybir
from concourse import bass_utils
from concourse.masks import make_identity

B = 4096
D = 128
N = 2 * B            # 8192 rows of z
NCORES = 8
ROWS = N // NCORES   # 1024 rows per core
NBLK = ROWS // 128   # 8 row-blocks per core
NT = 40              # local column tiles kept per core (5 block-columns)
COLS = NT * 128      # 5120 local columns
GRP = 8              # tiles per DMA / norm group
NGRP = NT // GRP     # 5 groups
CS_LO = 1024         # colsummed local columns [CS_LO, CS_HI)
CS_HI = 4096
INV_T = 10.0         # 1/temperature
MAGIC = 0x5F3759DF   # fp32 rsqrt seed

F32 = mybir.dt.float32
BF16 = mybir.dt.bfloat16
I32 = mybir.dt.int32
AX = mybir.AxisListType
AF = mybir.ActivationFunctionType
OP = mybir.AluOpType

# column chunks of the exp stream: (base, width)
CHUNKS = [(0, 2048), (2048, 2048), (4096, 1024)]


def _build() -> bass.Bass:
    nc = bacc.Bacc(None)
    z_in = nc.declare_dram_parameter("z", [128, NT * D], F32, isOutput=False)
    out_row = nc.declare_dram_parameter("rowsum", [128, NBLK], F32, isOutput=True)
    out_cs = nc.declare_dram_parameter("colsum", [1, CS_HI - CS_LO], F32, isOutput=True)
    out_pos = nc.declare_dram_parameter("possum", [128, 1], F32, isOutput=True)

    z_re = z_in.rearrange("p (n d) -> p n d", d=D)  # local row = n*128 + p

    with tile.TileContext(nc) as tc:
        with ExitStack() as ctx:
            persist = ctx.enter_context(tc.tile_pool(name="persist", bufs=1))
            work = ctx.enter_context(tc.tile_pool(name="work", bufs=2))
            psum = ctx.enter_context(tc.tile_pool(name="psum", bufs=2, space="PSUM"))

            ident = persist.tile([128, 128], BF16)
            make_identity(nc, ident)
            ones_col = persist.tile([128, 1], BF16)
            nc.vector.memset(ones_col, 1.0)
            b_neg10 = persist.tile([128, 1], F32)
            nc.vector.memset(b_neg10, -INV_T)
            # prime the exp table set while the input DMA streams
            prime = persist.tile([128, 1], F32)
            nc.scalar.activation(prime, b_neg10, AF.Exp, bias=b_neg10)

            z_sb = persist.tile([128, NT, D], F32)
            sq = persist.tile([128, NT, D], BF16)
            ss = persist.tile([128, NT], F32)
            rn = persist.tile([128, NT], F32)
            zn = persist.tile([128, NT, D], BF16)
            znT = persist.tile([128, COLS], BF16)
            ej = persist.tile([128, NBLK, COLS], BF16)
            acc = persist.tile([128, NBLK, 4], F32)
            cs_sb = persist.tile([1, CS_HI - CS_LO], F32)
            ri = persist.tile([128, NT], I32)
            rt = persist.tile([128, NT], F32)
            ry = persist.tile([128, NT], F32)
            nc.vector.memset(acc, 0.0)

            for g in range(NGRP):
                sl = slice(g * GRP, (g + 1) * GRP)
                nc.sync.dma_start(out=z_sb[:, sl, :], in_=z_re[:, sl, :])

            def norm_pre(g):  # ACT square (same table set as exp), DVE reduce
                sl = slice(g * GRP, (g + 1) * GRP)
                nc.scalar.activation(sq[:, sl, :], z_sb[:, sl, :], AF.Square)
                nc.vector.reduce_sum(ss[:, sl], sq[:, sl, :], axis=AX.X)

            def rsqrt(sl):  # DVE bit-trick + 2 Newton: rn[sl] = 1/sqrt(ss[sl])
                s, i, t, y = ss[:, sl], ri[:, sl], rt[:, sl], ry[:, sl]
                nc.vector.tensor_scalar(i, s.bitcast(I32), 1, None, OP.logical_shift_right)
                nc.vector.tensor_scalar(i, i, -1, MAGIC, OP.mult, OP.add)
                y0 = i.bitcast(F32)
                nc.vector.tensor_mul(t, y0, y0)
                nc.vector.tensor_mul(t, s, t)
                nc.vector.tensor_scalar(t, t, -0.5, 1.5, OP.mult, OP.add)
                nc.vector.tensor_mul(y, y0, t)
                nc.vector.tensor_mul(t, y, y)
                nc.vector.tensor_mul(t, s, t)
                nc.vector.tensor_scalar(t, t, -0.5, 1.5, OP.mult, OP.add)
                nc.vector.tensor_mul(rn[:, sl], y, t)

            def zn_scale(g):  # DVE scale to bf16
                sl = slice(g * GRP, (g + 1) * GRP)
                nc.vector.tensor_mul(
                    zn[:, sl, :],
                    z_sb[:, sl, :],
                    rn[:, sl].broadcast_to((128, GRP, D)),
                )

            def transpose_grp(g):  # PE transpose + DVE copy into znT
                tp = psum.tile([128, GRP * 128], BF16, tag="pp", name="tp")
                for q in range(GRP):
                    nc.tensor.transpose(
                        tp[:, q * 128 : (q + 1) * 128],
                        zn[:, g * GRP + q, :],
                        ident,
                    )
                nc.vector.tensor_copy(znT[:, g * 1024 : (g + 1) * 1024], tp)

            def emit_sub(b, base, cw, slot):
                pt = psum.tile([128, 2048], F32, tag="pp", name="pt")
                for s in range(cw // 512):
                    nc.tensor.matmul(
                        pt[:, s * 512 : (s + 1) * 512],
                        znT[:, b * 128 : (b + 1) * 128],
                        znT[:, base + s * 512 : base + (s + 1) * 512],
                        start=True,
                        stop=True,
                    )
                nc.scalar.activation(
                    ej[:, b, base : base + cw],
                    pt[:, :cw],
                    AF.Exp,
                    scale=INV_T,
                    bias=b_neg10,
                    accum_out=acc[:, b, slot : slot + 1],
                )

            def emit_colsum(off):
                cs = psum.tile([1, 512], F32, tag="pp", name="cs")
                for b in range(NBLK):
                    nc.tensor.matmul(
                        cs,
                        ones_col,
                        ej[:, b, off : off + 512],
                        start=(b == 0),
                        stop=(b == NBLK - 1),
                    )
                nc.vector.tensor_copy(cs_sb[:, off - CS_LO : off - CS_LO + 512], cs)

            # group 0 normalizes alone, then block 0's first 1024 columns
            # become the FIRST exp; the rest of the prologue (groups 1-4)
            # pipelines under the running exp stream
            norm_pre(0)
            rsqrt(slice(0, GRP))
            zn_scale(0)
            transpose_grp(0)
            emit_sub(0, 0, 1024, 0)
            norm_pre(1)
            rsqrt(slice(GRP, 2 * GRP))
            zn_scale(1)
            transpose_grp(1)
            emit_sub(0, 1024, 1024, 3)
            for b in range(1, NBLK):
                emit_sub(b, 0, 2048, 0)
                if b in (1, 3, 5):           # norm work under the exp stream
                    g = 2 + b // 2
                    norm_pre(g)
                    rsqrt(slice(g * GRP, (g + 1) * GRP))
                    zn_scale(g)
                if b in (2, 4, 6):           # PE transposes between slot waits
                    transpose_grp(2 + (b - 2) // 2)
            for b in range(NBLK):
                emit_sub(b, 2048, 2048, 1)
            # chunk-0 colsums run while ACT streams chunk-1 exps; chunk-1
            # colsums interleave with chunk-2 matmul fills (their ej regions
            # are complete before the in-order PE queue reaches them)
            emit_colsum(1024)
            emit_colsum(1536)
            for b in range(NBLK):
                emit_sub(b, 4096, 1024, 2)
                if b % 2 == 1:
                    emit_colsum(2048 + (b // 2) * 512)

            # ---- epilogue ------------------------------------------------
            # sum_r pos-cosine: zn rows 0..1023 dot zn rows 4096..5119
            pm = work.tile([128, NBLK, D], BF16, tag="pm", name="pm")
            nc.vector.tensor_mul(pm, zn[:, 0:NBLK, :], zn[:, 4 * NBLK : 5 * NBLK, :])
            pr = work.tile([128, NBLK], F32, tag="pr", name="pr")
            nc.vector.reduce_sum(pr, pm, axis=AX.X)
            possum = persist.tile([128, 1], F32)
            nc.vector.reduce_sum(possum, pr, axis=AX.X)
            nc.sync.dma_start(out=out_pos[:, :], in_=possum)

            rowsum = persist.tile([128, NBLK], F32)
            nc.vector.reduce_sum(rowsum, acc, axis=AX.X)
            nc.sync.dma_start(out=out_row[:, :], in_=rowsum)
            nc.sync.dma_start(out=out_cs[:, :], in_=cs_sb)

    nc.compile()
    return nc


_NC = None


def _get_nc() -> bass.Bass:
    global _NC
    if _NC is None:
        _NC = _build()
    return _NC


def make_in_maps(z: np.ndarray) -> list[dict]:
    maps = []
    for c in range(NCORES):
        zl = np.roll(z, -c * ROWS, axis=0)[:COLS]          # [5120, 128]
        zt = zl.reshape(NT, 128, D).transpose(1, 0, 2)     # [128, 40, 128]
        maps.append({"z": np.ascontiguousarray(zt.reshape(128, NT * D))})
    return maps


def kernel(emb0: np.ndarray, emb1: np.ndarray) -> np.ndarray:
    z = np.concatenate(
        [np.asarray(emb0, np.float32), np.asarray(emb1, np.float32)], axis=0
    )
    res = bass_utils.run_bass_kernel_spmd(
        _get_nc(), make_in_maps(z), core_ids=list(range(NCORES))
    )
    # assemble full row sums of exp(10 cos - 10) from per-core partials
    S = np.zeros(N, dtype=np.float64)
    pos_total = 0.0
    for c, r in enumerate(res.results):
        rows = r["rowsum"].astype(np.float64).T.reshape(ROWS)  # local row n*128+p
        S[c * ROWS : (c + 1) * ROWS] += rows
        idx = (c * ROWS + CS_LO + np.arange(CS_HI - CS_LO)) % N
        np.add.at(S, idx, r["colsum"].astype(np.float64).reshape(-1))
        pos_total += float(r["possum"].sum(dtype=np.float64))
    S -= 1.0  # remove the diagonal term exp(10*1 - 10) = 1
    total = float(np.sum(np.log(S))) + INV_T * N - INV_T * pos_total
    return np.asarray(np.float32(total / N))



# revision 4
# speedup vs baseline: 1.0229x; 1.0229x over previous
"""Trainium2 Bass kernel for nn_ContrastiveLoss (SimCLR-style, N=8192, D=128).

v5: host-normalized d-major input + DVE colsum/rowsum + quadrant half-pair.

Host normalizes z and ships znT = zn.T per core as bf16 [128(d), 5120 cols]
(10KB contiguous per partition -> ~256 total DMA descriptors over the two
HWDGE rings instead of v4's 640).  On device each core computes
e = exp(10 cos - 10) for its 8 stationary sub-blocks (own 1024 rows) times
5120 local columns:
  cols [0:1024]    own block (diag)        rowsum only
  cols [1024:4096] blocks c+1..c+3         rowsum + colsum
  cols [4096:5120] the {c,c+4} pair block, quadrant-split so each endpoint
                   computes half the pairs: sub-row k does cols
                   [4096+(k%2)*512, +512)  rowsum + colsum
The {c,c+4} quadrant split keeps one program for all cores: the host places
own rows interleaved (even subs = own[0:512], odd subs = own[512:1024]) and
picks the partner half-columns per core, so sets
  evens x H0  +  odds x H1   (this core)
  evens x H0  +  odds x H1   (partner core, complementary halves)
partition the 1024x1024 pair block exactly.  W = 36864 exp-cols/core vs
v4's 40960.

ACT is the critical engine: 24 exp instrs (8x2048 + 8x2048 + 8x512),
no accum reads (rowsums via DVE 4x-mode reduces over the persistent bf16
ej slabs, colsums via DVE bf16 adds into csacc + one final ones-matmul
pass).  Outputs are descriptor-light: cs [1,4096] (1 desc) and a
PE-transposed rs [8,128] (8 descs).
"""

import sys

sys.path.insert(0, "/opt/trn_rl_repo")

from contextlib import ExitStack

import numpy as np
import ml_dtypes

import concourse.bass as bass
import concourse.bacc as bacc
import concourse.tile as tile
from concourse import mybir
from concourse import bass_utils
from concourse.masks import make_identity

B = 4096
D = 128
N = 2 * B            # 8192 rows of z
NCORES = 8
ROWS = N // NCORES   # 1024 rows per core
NBLK = ROWS // 128   # 8 stationary sub-blocks per core
COLS = 5120          # local columns kept per core
HP = 4096            # half-pair region start
EJW = 4608           # per-sub-row ej width: 2048 + 2048 + 512
INV_T = 10.0         # 1/temperature

F32 = mybir.dt.float32
BF16 = mybir.dt.bfloat16
AX = mybir.AxisListType
AF = mybir.ActivationFunctionType
OP = mybir.AluOpType


def _build() -> bass.Bass:
    nc = bacc.Bacc(None)
    z_in = nc.declare_dram_parameter("z", [128, COLS], BF16, isOutput=False)
    out_cs = nc.declare_dram_parameter("cs", [1, 4096], F32, isOutput=True)
    out_rs = nc.declare_dram_parameter("rs", [NBLK, 128], F32, isOutput=True)

    with tile.TileContext(nc) as tc:
        with ExitStack() as ctx:
            persist = ctx.enter_context(tc.tile_pool(name="persist", bufs=1))
            psum = ctx.enter_context(tc.tile_pool(name="psum", bufs=2, space="PSUM"))

            znT = persist.tile([128, COLS], BF16)
            # input DMA split across the two HWDGE rings (sync + scalar)
            nc.sync.dma_start(out=znT[:, 0:2048], in_=z_in[:, 0:2048])
            nc.scalar.dma_start(out=znT[:, 2048:COLS], in_=z_in[:, 2048:COLS])

            b_neg10 = persist.tile([128, 1], F32)
            nc.vector.memset(b_neg10, -INV_T)
            ones_col = persist.tile([128, 1], BF16)
            nc.vector.memset(ones_col, 1.0)
            ident = persist.tile([128, 128], F32)
            make_identity(nc, ident)
            # prime the exp table set while the input DMA streams
            prime = persist.tile([128, 1], F32)
            nc.scalar.activation(prime, b_neg10, AF.Exp, bias=b_neg10)

            ej = persist.tile([128, NBLK, EJW], BF16)
            csacc = persist.tile([128, 4096], BF16)
            rs3 = persist.tile([128, NBLK, 3], F32)
            rs = persist.tile([128, NBLK], F32)
            cs_sb = persist.tile([1, 4096], F32)
            rs_sb = persist.tile([NBLK, 128], F32)

            def stat(b):
                return znT[:, b * 128:(b + 1) * 128]

            # ---- phase 0: cols [0:2048] (diag + first colsum chunk) ----
            for b in range(NBLK):
                pt = psum.tile([128, 2048], F32, tag="pp", name="pt")
                for s in range(4):
                    nc.tensor.matmul(
                        pt[:, s * 512:(s + 1) * 512],
                        stat(b),
                        znT[:, s * 512:(s + 1) * 512],
                        start=True,
                        stop=True,
                    )
                nc.scalar.activation(
                    ej[:, b, 0:2048], pt, AF.Exp, scale=INV_T, bias=b_neg10
                )
                if b == 0:
                    nc.vector.tensor_copy(csacc[:, 0:1024], ej[:, b, 1024:2048])
                else:
                    nc.vector.tensor_add(
                        csacc[:, 0:1024], csacc[:, 0:1024], ej[:, b, 1024:2048]
                    )
                nc.vector.reduce_sum(rs3[:, b, 0:1], ej[:, b, 0:2048], axis=AX.X)

            # ---- phase 1: cols [2048:4096] (colsum chunks 2,3) ----------
            for b in range(NBLK):
                pt = psum.tile([128, 2048], F32, tag="pp", name="pt")
                for s in range(4):
                    nc.tensor.matmul(
                        pt[:, s * 512:(s + 1) * 512],
                        stat(b),
                        znT[:, 2048 + s * 512:2048 + (s + 1) * 512],
                        start=True,
                        stop=True,
                    )
                nc.scalar.activation(
                    ej[:, b, 2048:4096], pt, AF.Exp, scale=INV_T, bias=b_neg10
                )
                if b == 0:
                    nc.vector.tensor_copy(csacc[:, 1024:3072], ej[:, b, 2048:4096])
                else:
                    nc.vector.tensor_add(
                        csacc[:, 1024:3072], csacc[:, 1024:3072], ej[:, b, 2048:4096]
                    )
                nc.vector.reduce_sum(rs3[:, b, 1:2], ej[:, b, 2048:4096], axis=AX.X)

            # ---- phase 2: half-pair cols + final colsum matmuls ---------
            def cs_emit(j):
                cpt = psum.tile([128, 2048], F32, tag="pp", name="cpt")
                cp = cpt[0:1, 0:512]
                nc.tensor.matmul(
                    cp, ones_col, csacc[:, j * 512:(j + 1) * 512],
                    start=True, stop=True,
                )
                nc.vector.tensor_copy(cs_sb[:, j * 512:(j + 1) * 512], cp)

            for b in range(NBLK):
                h = HP + (b % 2) * 512
                p2t = psum.tile([128, 2048], F32, tag="pp", name="p2t")
                p2 = p2t[:, 0:512]
                nc.tensor.matmul(p2, stat(b), znT[:, h:h + 512], start=True, stop=True)
                nc.scalar.activation(
                    ej[:, b, 4096:4608], p2, AF.Exp, scale=INV_T, bias=b_neg10
                )
                co = 3072 + (b % 2) * 512
                if b < 2:
                    nc.vector.tensor_copy(csacc[:, co:co + 512], ej[:, b, 4096:4608])
                else:
                    nc.vector.tensor_add(
                        csacc[:, co:co + 512], csacc[:, co:co + 512],
                        ej[:, b, 4096:4608],
                    )
                nc.vector.reduce_sum(rs3[:, b, 2:3], ej[:, b, 4096:4608], axis=AX.X)
                if 1 <= b <= 6:          # csacc[0:3072] complete after phase 1
                    cs_emit(b - 1)

            # ---- tail ---------------------------------------------------
            cs_emit(6)  # hp even half (complete after b=6's add)
            cs_emit(7)  # hp odd half (complete after b=7's add)
            nc.vector.reduce_sum(rs, rs3, axis=AX.X)
            rstt = psum.tile([128, 2048], F32, tag="pp", name="rstt")
            rst = rstt[0:NBLK, 0:128]
            nc.tensor.transpose(rst, rs, ident)
            nc.vector.tensor_copy(rs_sb, rst)
            nc.sync.dma_start(out=out_rs[:, :], in_=rs_sb)
            nc.sync.dma_start(out=out_cs[:, :], in_=cs_sb)

    nc.compile()
    return nc


_NC = None


def _get_nc() -> bass.Bass:
    global _NC
    if _NC is None:
        _NC = _build()
    return _NC


def _base_k(k: int) -> int:
    return (k // 2) * 128 + (512 if k % 2 else 0)


def make_in_maps(zn: np.ndarray) -> list[dict]:
    """zn: [8192, 128] float32, already L2-normalized."""
    zn16 = zn.astype(ml_dtypes.bfloat16)
    maps = []
    for c in range(NCORES):
        own = c * ROWS
        cols = []
        for k in range(NBLK):
            cols.append(own + _base_k(k) + np.arange(128))
        cols.append((own + 1024 + np.arange(3072)) % N)
        p = ((c + 4) % 8) * ROWS
        if c < 4:
            cols.append(p + np.arange(512))
            cols.append(p + 512 + np.arange(512))
        else:
            cols.append(p + 512 + np.arange(512))
            cols.append(p + np.arange(512))
        idx = np.concatenate(cols)
        znT = np.ascontiguousarray(zn16[idx].T)   # [128, 5120]
        maps.append({"z": znT})
    return maps


def kernel(emb0: np.ndarray, emb1: np.ndarray) -> np.ndarray:
    z = np.concatenate(
        [np.asarray(emb0, np.float32), np.asarray(emb1, np.float32)], axis=0
    )
    nrm = np.maximum(np.sqrt((z * z).sum(axis=1, keepdims=True)), 1e-8)
    zn = z / nrm
    res = bass_utils.run_bass_kernel_spmd(
        _get_nc(), make_in_maps(zn), core_ids=list(range(NCORES))
    )
    # assemble full row sums of exp(10 cos - 10) from per-core partials
    S = np.zeros(N, dtype=np.float64)
    for c, r in enumerate(res.results):
        rs = r["rs"].astype(np.float64)           # [8, 128]
        cs = r["cs"].astype(np.float64).reshape(-1)  # [4096]
        for k in range(NBLK):
            S[c * ROWS + _base_k(k): c * ROWS + _base_k(k) + 128] += rs[k]
        idx = (c * ROWS + 1024 + np.arange(3072)) % N
        np.add.at(S, idx, cs[0:3072])
        p = ((c + 4) % 8) * ROWS
        if c < 4:
            S[p:p + 512] += cs[3072:3584]
            S[p + 512:p + 1024] += cs[3584:4096]
        else:
            S[p + 512:p + 1024] += cs[3072:3584]
            S[p:p + 512] += cs[3584:4096]
    S -= 1.0  # remove the diagonal term exp(10*1 - 10) = 1
    g_pos = np.einsum("ij,ij->i", zn, np.roll(zn, -B, axis=0)).sum(dtype=np.float64)
    total = float(np.sum(np.log(S))) + INV_T * N - INV_T * g_pos
    return np.asarray(np.float32(total / N))


# revision 11
# speedup vs baseline: 1.2792x; 1.2505x over previous
"""Trainium2 Bass kernel for nn_ContrastiveLoss (SimCLR-style, N=8192, D=128).

v5: host-normalized d-major input + DVE colsum/rowsum + quadrant half-pair.

Host normalizes z and ships znT = zn.T per core as bf16 [128(d), 5120 cols]
(10KB contiguous per partition -> ~256 total DMA descriptors over the two
HWDGE rings instead of v4's 640).  On device each core computes
e = exp(10 cos - 10) for its 8 stationary sub-blocks (own 1024 rows) times
5120 local columns:
  cols [0:1024]    own block (diag)        rowsum only
  cols [1024:4096] blocks c+1..c+3         rowsum + colsum
  cols [4096:5120] the {c,c+4} pair block, quadrant-split so each endpoint
                   computes half the pairs: sub-row k does cols
                   [4096+(k%2)*512, +512)  rowsum + colsum
The {c,c+4} quadrant split keeps one program for all cores: the host places
own rows interleaved (even subs = own[0:512], odd subs = own[512:1024]) and
picks the partner half-columns per core, so sets
  evens x H0  +  odds x H1   (this core)
  evens x H0  +  odds x H1   (partner core, complementary halves)
partition the 1024x1024 pair block exactly.  W = 36864 exp-cols/core vs
v4's 40960.

ACT is the critical engine: 24 exp instrs (8x2048 + 8x2048 + 8x512),
no accum reads (rowsums via DVE 4x-mode reduces over the persistent bf16
ej slabs, colsums via DVE bf16 adds into csacc + one final ones-matmul
pass).  Outputs are descriptor-light: cs [1,4096] (1 desc) and a
PE-transposed rs [8,128] (8 descs).
"""

import sys

sys.path.insert(0, "/opt/trn_rl_repo")

from contextlib import ExitStack

import numpy as np
import ml_dtypes

import concourse.bass as bass
import concourse.bacc as bacc
import concourse.tile as tile
from concourse import mybir
from concourse import bass_utils
from concourse.masks import make_identity

B = 4096
D = 128
N = 2 * B            # 8192 rows of z
NCORES = 8
ROWS = N // NCORES   # 1024 rows per core
NBLK = ROWS // 128   # 8 stationary sub-blocks per core
COLS = 5120          # local columns kept per core
HP = 4096            # half-pair region start
EJW = 4608           # per-sub-row ej width: 2048 + 2048 + 512
INV_T = 10.0         # 1/temperature

F32 = mybir.dt.float32
BF16 = mybir.dt.bfloat16
AX = mybir.AxisListType
AF = mybir.ActivationFunctionType
OP = mybir.AluOpType


def _build() -> bass.Bass:
    nc = bacc.Bacc(None)
    z_in = nc.declare_dram_parameter("z", [128, COLS], BF16, isOutput=False)
    out_cs = nc.declare_dram_parameter("cs", [1, 4096], F32, isOutput=True)
    out_rs = nc.declare_dram_parameter("rs", [NBLK, 128], F32, isOutput=True)

    with tile.TileContext(nc) as tc:
        with ExitStack() as ctx:
            persist = ctx.enter_context(tc.tile_pool(name="persist", bufs=1))
            psum = ctx.enter_context(tc.tile_pool(name="psum", bufs=2, space="PSUM"))

            znT = persist.tile([128, COLS], BF16)
            # input DMA split across the two HWDGE rings (sync + scalar)
            nc.sync.dma_start(out=znT[:, 0:2048], in_=z_in[:, 0:2048])
            nc.scalar.dma_start(out=znT[:, 2048:COLS], in_=z_in[:, 2048:COLS])

            b_neg10 = persist.tile([128, 1], F32)
            nc.vector.memset(b_neg10, -INV_T)
            ones_col = persist.tile([128, 1], BF16)
            nc.vector.memset(ones_col, 1.0)
            ident = persist.tile([128, 128], F32)
            make_identity(nc, ident)
            # prime the exp table set while the input DMA streams
            prime = persist.tile([128, 1], F32)
            nc.scalar.activation(prime, b_neg10, AF.Exp, bias=b_neg10)

            ej = persist.tile([128, NBLK, EJW], BF16)
            csacc = persist.tile([128, 4096], BF16)
            acc = persist.tile([128, NBLK, 2], F32)
            rs3 = persist.tile([128, NBLK], F32)
            rs_a = persist.tile([128, NBLK], F32)
            rs = persist.tile([128, NBLK], F32)
            cs_sb = persist.tile([1, 4096], F32)
            rs_sb = persist.tile([NBLK, 128], F32)

            def stat(b):
                return znT[:, b * 128:(b + 1) * 128]

            # ---- phase 0: cols [0:2048] (diag + first colsum chunk) ----
            for b in range(NBLK):
                pt = psum.tile([128, 2048], F32, tag="pp", name="pt")
                for s in range(4):
                    nc.tensor.matmul(
                        pt[:, s * 512:(s + 1) * 512],
                        stat(b),
                        znT[:, s * 512:(s + 1) * 512],
                        start=True,
                        stop=True,
                    )
                nc.scalar.activation(
                    ej[:, b, 0:2048], pt, AF.Exp, scale=INV_T, bias=b_neg10,
                    accum_out=acc[:, b, 0:1],
                )
                if b == 0:
                    nc.vector.tensor_copy(csacc[:, 0:1024], ej[:, b, 1024:2048])
                else:
                    nc.vector.tensor_add(
                        csacc[:, 0:1024], csacc[:, 0:1024], ej[:, b, 1024:2048]
                    )

            # ---- phase 1: cols [2048:4096] (colsum chunks 2,3) ----------
            for b in range(NBLK):
                pt = psum.tile([128, 2048], F32, tag="pp", name="pt")
                for s in range(4):
                    nc.tensor.matmul(
                        pt[:, s * 512:(s + 1) * 512],
                        stat(b),
                        znT[:, 2048 + s * 512:2048 + (s + 1) * 512],
                        start=True,
                        stop=True,
                    )
                nc.scalar.activation(
                    ej[:, b, 2048:4096], pt, AF.Exp, scale=INV_T, bias=b_neg10,
                    accum_out=acc[:, b, 1:2],
                )
                if b == 0:
                    nc.vector.tensor_copy(csacc[:, 1024:3072], ej[:, b, 2048:4096])
                else:
                    nc.vector.tensor_add(
                        csacc[:, 1024:3072], csacc[:, 1024:3072], ej[:, b, 2048:4096]
                    )

            # ---- phase 2: half-pair cols + final colsum matmuls ---------
            def cs_emit(j):
                cpt = psum.tile([128, 2048], F32, tag="pp", name="cpt")
                cp = cpt[0:1, 0:512]
                nc.tensor.matmul(
                    cp, ones_col, csacc[:, j * 512:(j + 1) * 512],
                    start=True, stop=True,
                )
                nc.vector.tensor_copy(cs_sb[:, j * 512:(j + 1) * 512], cp)

            for b in range(NBLK):
                h = HP + (b % 2) * 512
                p2t = psum.tile([128, 2048], F32, tag="pp", name="p2t")
                p2 = p2t[:, 0:512]
                nc.tensor.matmul(p2, stat(b), znT[:, h:h + 512], start=True, stop=True)
                nc.scalar.activation(
                    ej[:, b, 4096:4608], p2, AF.Exp, scale=INV_T, bias=b_neg10
                )
                co = 3072 + (b % 2) * 512
                if b < 2:
                    nc.vector.tensor_copy(csacc[:, co:co + 512], ej[:, b, 4096:4608])
                else:
                    nc.vector.tensor_add(
                        csacc[:, co:co + 512], csacc[:, co:co + 512],
                        ej[:, b, 4096:4608],
                    )
                nc.vector.reduce_sum(rs3[:, b:b + 1], ej[:, b, 4096:4608], axis=AX.X)
                if 1 <= b <= 6:          # csacc[0:3072] complete after phase 1
                    cs_emit(b - 1)

            # ---- tail ---------------------------------------------------
            cs_emit(6)  # hp even half (complete after b=6's add)
            cs_emit(7)  # hp odd half (complete after b=7's add)
            nc.vector.reduce_sum(rs_a, acc, axis=AX.X)
            nc.vector.tensor_add(rs, rs_a, rs3)
            rstt = psum.tile([128, 2048], F32, tag="pp", name="rstt")
            rst = rstt[0:NBLK, 0:128]
            nc.tensor.transpose(rst, rs, ident)
            nc.vector.tensor_copy(rs_sb, rst)
            nc.sync.dma_start(out=out_rs[:, :], in_=rs_sb)
            nc.sync.dma_start(out=out_cs[:, :], in_=cs_sb)

    nc.compile()
    return nc


_NC = None


def _get_nc() -> bass.Bass:
    global _NC
    if _NC is None:
        _NC = _build()
    return _NC


def _base_k(k: int) -> int:
    return (k // 2) * 128 + (512 if k % 2 else 0)


def make_in_maps(zn: np.ndarray) -> list[dict]:
    """zn: [8192, 128] float32, already L2-normalized."""
    zn16 = zn.astype(ml_dtypes.bfloat16)
    maps = []
    for c in range(NCORES):
        own = c * ROWS
        cols = []
        for k in range(NBLK):
            cols.append(own + _base_k(k) + np.arange(128))
        cols.append((own + 1024 + np.arange(3072)) % N)
        p = ((c + 4) % 8) * ROWS
        if c < 4:
            cols.append(p + np.arange(512))
            cols.append(p + 512 + np.arange(512))
        else:
            cols.append(p + 512 + np.arange(512))
            cols.append(p + np.arange(512))
        idx = np.concatenate(cols)
        znT = np.ascontiguousarray(zn16[idx].T)   # [128, 5120]
        maps.append({"z": znT})
    return maps


def kernel(emb0: np.ndarray, emb1: np.ndarray) -> np.ndarray:
    z = np.concatenate(
        [np.asarray(emb0, np.float32), np.asarray(emb1, np.float32)], axis=0
    )
    nrm = np.maximum(np.sqrt((z * z).sum(axis=1, keepdims=True)), 1e-8)
    zn = z / nrm
    res = bass_utils.run_bass_kernel_spmd(
        _get_nc(), make_in_maps(zn), core_ids=list(range(NCORES))
    )
    # assemble full row sums of exp(10 cos - 10) from per-core partials
    S = np.zeros(N, dtype=np.float64)
    for c, r in enumerate(res.results):
        rs = r["rs"].astype(np.float64)           # [8, 128]
        cs = r["cs"].astype(np.float64).reshape(-1)  # [4096]
        for k in range(NBLK):
            S[c * ROWS + _base_k(k): c * ROWS + _base_k(k) + 128] += rs[k]
        idx = (c * ROWS + 1024 + np.arange(3072)) % N
        np.add.at(S, idx, cs[0:3072])
        p = ((c + 4) % 8) * ROWS
        if c < 4:
            S[p:p + 512] += cs[3072:3584]
            S[p + 512:p + 1024] += cs[3584:4096]
        else:
            S[p + 512:p + 1024] += cs[3072:3584]
            S[p:p + 512] += cs[3584:4096]
    S -= 1.0  # remove the diagonal term exp(10*1 - 10) = 1
    g_pos = np.einsum("ij,ij->i", zn, np.roll(zn, -B, axis=0)).sum(dtype=np.float64)
    total = float(np.sum(np.log(S))) + INV_T * N - INV_T * g_pos
    return np.asarray(np.float32(total / N))
